# revision 17
# baseline (speedup 1.0000x reference)
"""5-layer DGL-style GraphConv (AwA2Conv) on 8 Trainium2 NeuronCores — v3.

Math per layer (norm='both'):
    out = D_in^{-1/2} A D_out^{-1/2} (h) @ W + b     (+ leaky_relu except last)

Per-edge weight w_e = dinv_out[src]*dinv_in[dst] folded into block-sparse
"S" chunks (128-slot x 128-dst); aggregation = PE matmuls over gathered
edge rows, at width min(Fin, Fout) per layer. Aggregated activations live
feature-major in a resident flat SBUF scratch so the following dense
matmul reads them directly.

Perf structure (bottlenecks, in order): GpSimd dma_gather descriptor
generation is ~6.7ns/index and strictly serial — it bounds each
aggregation layer; the 2 AllGathers per exchange serialize on the
collective engine; HBM traffic is the third constraint.
- gathers merged to <=1024 indices per call (>=1040 wedges the device)
- S matrices (identical across layers) loaded ONCE into resident SBUF
- xg (layer-1 pregathered rows) shipped at true width 300, partition-major
- dense outputs split per node-half so AG_a fires early; lo-sweep
  aggregation stages into SBUF so all table-A work overlaps AG_b
- dense layers chase aggregation tiles; L1 aggregates straight into PSUM
  (both halves available immediately — no staging)
"""

import os

import numpy as np
import ml_dtypes

# recover cleanly if a previous run left the NeuronCores wedged
os.environ.setdefault("NEURON_RT_RESET_CORES", "1")

import concourse.bass as bass
import concourse.bacc as bacc
import concourse.mybir as mybir
import concourse.tile as tile
from concourse.bass_utils import run_bass_kernel_spmd

N_NODES = 50000
N_EDGES = 250000
NC = 8
NPC = N_NODES // NC      # 6250 nodes per core
HALF = 25000             # lo/hi source split (table A/B)
HPC = HALF // NC         # 3125 nodes per core per half
P = 128
TPH = 25                 # dst tiles per half (24x128 + 1x53)
N_TILES = 2 * TPH        # 50 dst tiles per core
DIMS = [300, 1024, 512, 256, 128, 2048]
NEG_SLOPE = 0.01
GRAN = 128               # slot granularity (chunk-aligned segments)
GCAP = 1024              # dma_gather hard cap on num_idxs

F32 = mybir.dt.float32
BF16 = mybir.dt.bfloat16
DT = BF16
NPDT = ml_dtypes.bfloat16
I16 = mybir.dt.int16
LRELU = mybir.ActivationFunctionType.Lrelu

H4OFF = 2 * NPC          # h4 node-major stage overlay offset in scratch


def _ceil_div(a, b):
    return (a + b - 1) // b


def _tile_start(t):
    return (t // TPH) * HPC + (t % TPH) * P


def _tile_width(t):
    return HPC - (TPH - 1) * P if (t % TPH) == TPH - 1 else P


TILE_STARTS = [_tile_start(t) for t in range(N_TILES)]
TILE_WIDTHS = [_tile_width(t) for t in range(N_TILES)]


def _make_groups(sched):
    """Pack consecutive tiles into gather groups of <= GCAP slots."""
    groups = []
    cur, cur_slots, s0 = [], 0, 0
    off = 0
    for t in range(N_TILES):
        s = int(sched[t])
        if cur and cur_slots + s > GCAP:
            groups.append((cur, s0, cur_slots))
            cur, cur_slots, s0 = [], 0, off
        cur.append(t)
        cur_slots += s
        off += s
    if cur:
        groups.append((cur, s0, cur_slots))
    assert all(s <= GCAP for _, _, s in groups)
    return groups


# ----------------------------------------------------------------------------
# Host-side graph preprocessing
# ----------------------------------------------------------------------------

def _prep(edge_index, x):
    src = np.asarray(edge_index[0], dtype=np.int64)
    dst = np.asarray(edge_index[1], dtype=np.int64)
    out_deg = np.bincount(src, minlength=N_NODES).astype(np.float32)
    in_deg = np.bincount(dst, minlength=N_NODES).astype(np.float32)
    dinv_out = 1.0 / np.sqrt(np.maximum(out_deg, 1.0))
    dinv_in = 1.0 / np.sqrt(np.maximum(in_deg, 1.0))
    w = (dinv_out[src] * dinv_in[dst]).astype(np.float32)
    xb = np.asarray(x, dtype=np.float32)

    d_half = dst // HALF
    d_rem = dst % HALF
    d_core = d_rem // HPC
    d_with = d_rem % HPC
    d_pos = d_with + d_half * HPC
    d_tile = d_half * TPH + np.minimum(d_with // P, TPH - 1)
    hi = (src >= HALF).astype(np.int64)

    key = (d_core * N_TILES + d_tile) * 2 + hi
    order = np.lexsort((src, key))
    src_s, w_s, pos_s, key_s = src[order], w[order], d_pos[order], key[order]
    bounds = np.searchsorted(key_s, np.arange(NC * N_TILES * 2 + 1))

    n_lo = np.zeros((NC, N_TILES), dtype=np.int64)
    n_hi = np.zeros((NC, N_TILES), dtype=np.int64)
    for c in range(NC):
        for t in range(N_TILES):
            k = (c * N_TILES + t) * 2
            n_lo[c, t] = bounds[k + 1] - bounds[k]
            n_hi[c, t] = bounds[k + 2] - bounds[k + 1]

    sched_lo = np.maximum(
        np.ceil(n_lo.max(axis=0) / GRAN).astype(np.int64), 1) * GRAN
    sched_hi = np.maximum(
        np.ceil(n_hi.max(axis=0) / GRAN).astype(np.int64), 1) * GRAN

    soff = {"lo": np.concatenate([[0], np.cumsum(sched_lo)]).astype(int),
            "hi": np.concatenate([[0], np.cumsum(sched_hi)]).astype(int)}
    scheds = {"lo": sched_lo, "hi": sched_hi}

    per_core = []
    for c in range(NC):
        pc = {}
        for nm, hbit in (("lo", 0), ("hi", 1)):
            slots = int(soff[nm][-1])
            nch = slots // P
            idx_flat = np.zeros(slots, dtype=np.int16)
            s_flat = np.zeros((slots, P), dtype=np.float32)
            xg_flat = np.zeros((slots, 300), dtype=NPDT)
            for t in range(N_TILES):
                k = (c * N_TILES + t) * 2 + hbit
                a, b_ = int(bounds[k]), int(bounds[k + 1])
                ne = b_ - a
                j0 = int(soff[nm][t])
                assert ne <= int(scheds[nm][t])
                if ne == 0:
                    continue
                srcs = src_s[a:b_] - hbit * HALF
                idx_flat[j0 : j0 + ne] = srcs.astype(np.int16)
                cols = (pos_s[a:b_] - TILE_STARTS[t]).astype(np.int64)
                s_flat[j0 + np.arange(ne), cols] = w_s[a:b_]
                xg_flat[j0 : j0 + ne, :] = xb[src_s[a:b_]].astype(NPDT)
            pc[f"idx_{nm}"] = np.ascontiguousarray(
                np.tile(idx_flat.reshape(-1, 16).T, (8, 1)))
            pc[f"s_{nm}"] = np.ascontiguousarray(
                s_flat.reshape(nch, P, P).transpose(1, 0, 2).astype(NPDT))
            pc[f"xg_{nm}"] = np.ascontiguousarray(
                xg_flat.reshape(nch, P, 300).transpose(1, 0, 2))
        per_core.append(pc)
    return sched_lo, sched_hi, per_core


# ----------------------------------------------------------------------------
# Bass program builder (depends only on the schedules)
# ----------------------------------------------------------------------------

def _build(sched_lo, sched_hi):
    nc = bacc.Bacc("TRN2")
    scheds = {"lo": sched_lo, "hi": sched_hi}
    coff = {nm: np.concatenate([[0], np.cumsum(scheds[nm] // P)]).astype(int)
            for nm in ("lo", "hi")}
    ioff = {nm: np.concatenate([[0], np.cumsum(scheds[nm] // 16)]).astype(int)
            for nm in ("lo", "hi")}
    groups = {nm: _make_groups(scheds[nm]) for nm in ("lo", "hi")}
    totc = {nm: int(coff[nm][-1]) for nm in ("lo", "hi")}
    toti = {nm: int(ioff[nm][-1]) for nm in ("lo", "hi")}

    xg_lo_d = nc.declare_dram_parameter("xg_lo", [128, totc["lo"], 300], DT, isOutput=False)
    xg_hi_d = nc.declare_dram_parameter("xg_hi", [128, totc["hi"], 300], DT, isOutput=False)
    s_lo_d = nc.declare_dram_parameter("s_lo", [128, totc["lo"], P], DT, isOutput=False)
    s_hi_d = nc.declare_dram_parameter("s_hi", [128, totc["hi"], P], DT, isOutput=False)
    idx_lo_d = nc.declare_dram_parameter("idx_lo", [128, toti["lo"]], I16, isOutput=False)
    idx_hi_d = nc.declare_dram_parameter("idx_hi", [128, toti["hi"]], I16, isOutput=False)
    Ws, bs = [], []
    for i in range(5):
        fi, fo = DIMS[i], DIMS[i + 1]
        Ws.append(nc.declare_dram_parameter(f"W{i+1}", [fi, fo], DT, isOutput=False))
        bs.append(nc.declare_dram_parameter(f"b{i+1}", [fo, 1], F32, isOutput=False))
    b4r_d = nc.declare_dram_parameter("b4r", [1, 128], DT, isOutput=False)
    b5r_d = nc.declare_dram_parameter("b5r", [128, 2048], DT, isOutput=False)
    out_d = nc.declare_dram_parameter("out", [NPC, 2048], DT, isOutput=True)

    with tile.TileContext(nc) as tc:
        with (
            tc.tile_pool(name="dram", bufs=1, space="DRAM") as dram,
            tc.tile_pool(name="cpool", bufs=1) as cpool,
            tc.tile_pool(name="sb", bufs=2) as sb,
            tc.tile_pool(name="pagg", bufs=1, space="PSUM") as pagg,
            tc.tile_pool(name="pmm", bufs=4, space="PSUM") as pmm,
        ):
            # ---- internal DRAM ----
            h1T_a = dram.tile([128, 8 * HPC], DT)
            h1T_b = dram.tile([128, 8 * HPC], DT)
            g2a = dram.tile([HPC, 512], DT)
            g2b = dram.tile([HPC, 512], DT)
            g3a = dram.tile([HPC, 256], DT)
            g3b = dram.tile([HPC, 256], DT)
            g4a = dram.tile([HPC, 128], DT)
            g4b = dram.tile([HPC, 128], DT)
            h4a = dram.tile([HPC, 128], DT)
            h4b = dram.tile([HPC, 128], DT)
            T2a = dram.tile([HALF, 512], DT, addr_space="Shared")
            T2b = dram.tile([HALF, 512], DT, addr_space="Shared")
            T3a = dram.tile([HALF, 256], DT, addr_space="Shared")
            T3b = dram.tile([HALF, 256], DT, addr_space="Shared")
            T4a = dram.tile([HALF, 128], DT, addr_space="Shared")
            T4b = dram.tile([HALF, 128], DT, addr_space="Shared")
            T5a = dram.tile([HALF, 128], DT, addr_space="Shared")
            T5b = dram.tile([HALF, 128], DT, addr_space="Shared")

            # ---- resident SBUF ----
            # flat scratch: chunks k at cols [k*NPC,(k+1)*NPC); h4 node-major
            # stage overlays cols [H4OFF, H4OFF + 50*128) (chunks 2-3 region,
            # free during L4 which only reads chunks 0-1)
            scratch = cpool.tile([P, 4 * NPC], DT, name="scratch")
            s_sb = {}
            s_sb["lo"] = cpool.tile([128, totc["lo"], P], DT, name="slores")
            nc.sync.dma_start(s_sb["lo"][:], s_lo_d[:])
            s_sb["hi"] = cpool.tile([128, totc["hi"], P], DT, name="shires")
            nc.sync.dma_start(s_sb["hi"][:], s_hi_d[:])
            ones_sb = cpool.tile([1, 128], DT, name="ones")
            nc.any.memset(ones_sb[:], 1.0)
            b4r_sb = cpool.tile([1, 128], DT, name="b4rsb")
            nc.sync.dma_start(b4r_sb[:], b4r_d[:])
            b5r_sb = cpool.tile([128, 2048], DT, name="b5rsb")
            nc.sync.dma_start(b5r_sb[:], b5r_d[:])
            idx_sb = {}
            idx_sb["lo"] = cpool.tile([128, toti["lo"]], I16, name="idxlo")
            nc.sync.dma_start(idx_sb["lo"][:], idx_lo_d[:])
            idx_sb["hi"] = cpool.tile([128, toti["hi"]], I16, name="idxhi")
            nc.sync.dma_start(idx_sb["hi"][:], idx_hi_d[:])

            rg = [list(range(NC))]

            def load_w(i):
                fi, fo = DIMS[i], DIMS[i + 1]
                ks = []
                for k in range(_ceil_div(fi, P)):
                    kk = min(P, fi - k * P)
                    t_ = cpool.tile([P, fo], DT, name=f"w{i}_{k}")
                    nc.sync.dma_start(t_[:kk, :], Ws[i][k * P : k * P + kk, :])
                    ks.append((t_, kk))
                return ks

            def load_bcol(i):
                fo = DIMS[i + 1]
                t_ = cpool.tile([P, 16], F32, name=f"bc{i}")
                for m in range(_ceil_div(fo, P)):
                    mm = min(P, fo - m * P)
                    nc.sync.dma_start(t_[:mm, m : m + 1], bs[i][m * P : m * P + mm, :])
                return t_

            w1 = load_w(0)
            w2 = load_w(1)
            w3 = load_w(2)
            w4 = load_w(3)
            w5 = load_w(4)
            b1c = load_bcol(0)
            b2c = load_bcol(1)
            b3c = load_bcol(2)

            def ag(src_d, dst_d):
                nc.gpsimd.collective_compute(
                    "AllGather", mybir.AluOpType.bypass, replica_groups=rg,
                    ins=[src_d[:].opt()], outs=[dst_d[:].opt()],
                )

            def sc(k, c0, c1, pw=P):
                return scratch[:pw, k * NPC + c0 : k * NPC + c1]

            # ---------------- aggregation sweeps (layers 2-5) ----------------
            def sweep(layer, nm, tab, fa, mode, group_cb=None):
                """One half-sweep of a layer's aggregation via dma_gather.

                mode 'copy': psum -> stage; 'add': stage += psum.
                Stage is scratch chunks (feature-major) or the h4 overlay
                (layer 3, node-major). group_cb(last_tile) after each group.
                """
                nfc = _ceil_div(fa, P)
                sched = scheds[nm]
                ssb = s_sb[nm]
                for tiles, s0, slots in groups[nm]:
                    t0, t1 = tiles[0], tiles[-1]
                    c0 = int(coff[nm][t0])
                    gc = int(coff[nm][t1 + 1]) - c0
                    hg = sb.tile([128, gc, fa], DT, name=f"hg_{layer}_{nm}_{t0}", tag="hg", bufs=3)
                    nc.gpsimd.dma_gather(
                        hg[:], tab,
                        idx_sb[nm][:, int(ioff[nm][t0]) : int(ioff[nm][t1 + 1])],
                        slots, slots, fa,
                    )
                    for t in tiles:
                        tw = TILE_WIDTHS[t]
                        ts = TILE_STARTS[t]
                        ca = int(coff[nm][t]) - c0
                        cg = int(coff[nm][t])
                        nch = int(sched[t]) // P
                        if layer == 3:  # node-major
                            pt = pagg.tile([P, P], F32, name=f"pt_{layer}_{nm}_{t}",
                                           tag="pagg0", space="PSUM", bufs=1)
                            last = nch - 1 if mode == "copy" else nch
                            for ci in range(nch):
                                nc.tensor.matmul(
                                    pt[:, :fa], ssb[:, cg + ci, :], hg[:, ca + ci, :fa],
                                    start=(ci == 0), stop=(ci == last),
                                )
                            if mode == "add":  # bias row closes the group
                                nc.tensor.matmul(
                                    pt[:, :fa], ones_sb[:1, :], b4r_sb[:1, :fa],
                                    start=False, stop=True,
                                )
                            dst = scratch[:tw, H4OFF + t * P : H4OFF + t * P + fa]
                            if mode == "copy":
                                nc.vector.tensor_copy(dst, pt[:tw, :fa])
                            else:
                                nc.vector.tensor_tensor(
                                    out=dst, in0=pt[:tw, :fa], in1=dst,
                                    op=mybir.AluOpType.add,
                                )
                        else:
                            for fc in range(nfc):
                                fw = min(P, fa - fc * P)
                                pt = pagg.tile([P, P], F32, name=f"pt_{layer}_{nm}_{t}_{fc}",
                                               tag=f"pagg{fc}", space="PSUM", bufs=1)
                                for ci in range(nch):
                                    nc.tensor.matmul(
                                        pt[:fw, :],
                                        hg[:, ca + ci, fc * P : fc * P + fw],
                                        ssb[:, cg + ci, :],
                                        start=(ci == 0), stop=(ci == nch - 1),
                                    )
                                dst = sc(fc, ts, ts + tw, fw)
                                if mode == "copy":
                                    nc.vector.tensor_copy(dst, pt[:fw, :tw])
                                else:
                                    nc.vector.tensor_tensor(
                                        out=dst, in0=pt[:fw, :tw], in1=dst,
                                        op=mybir.AluOpType.add,
                                    )
                    if group_cb is not None:
                        group_cb(t1)

            # ---------------- dense helpers ----------------
            def dense_from_scratch(li, fi, fo, wt, ga, gb):
                """g[n,:] = scratch[:, :, n]^T @ W, evicted to ga/gb by half."""
                nk = _ceil_div(fi, P)
                state = {"done": 0}

                def emit_block(d0):
                    dw = min(512, NPC - d0)
                    for m4 in range(_ceil_div(dw, P)):
                        mw = min(P, dw - m4 * P)
                        r0 = d0 + m4 * P
                        pm = pmm.tile([P, fo], F32, name=f"pm_{li}_{d0}_{m4}",
                                      tag="pmm", space="PSUM")
                        for k in range(nk):
                            kk = min(P, fi - k * P)
                            nc.tensor.matmul(
                                pm[:mw, :fo],
                                sc(k, r0, r0 + mw, kk),
                                wt[k][0][:kk, :fo],
                                start=(k == 0), stop=(k == nk - 1),
                            )
                        ev = sb.tile([P, fo], DT, name=f"ev_{li}_{d0}_{m4}", tag="ev", bufs=6)
                        nc.vector.tensor_copy(ev[:mw, :fo], pm[:mw, :fo])
                        if r0 + mw <= HPC:
                            nc.scalar.dma_start(ga[r0 : r0 + mw, :fo], ev[:mw, :fo])
                        elif r0 >= HPC:
                            nc.scalar.dma_start(gb[r0 - HPC : r0 - HPC + mw, :fo], ev[:mw, :fo])
                        else:
                            cut = HPC - r0
                            nc.scalar.dma_start(ga[r0 : HPC, :fo], ev[:cut, :fo])
                            nc.scalar.dma_start(gb[0 : r0 + mw - HPC, :fo], ev[cut:mw, :fo])

                def cb(covered, on_a=None, on_b=None):
                    nblk = covered // 512 if covered < NPC else _ceil_div(NPC, 512)
                    while state["done"] < nblk:
                        emit_block(state["done"] * 512)
                        state["done"] += 1
                        if state["done"] * 512 >= HPC and on_a is not None:
                            on_a()
                            on_a = None
                    if covered >= NPC and on_b is not None:
                        on_b()
                    return on_a

                return cb

            # ================= L1: agg(x) -> scratch; dense W1 -> h1T =================
            def l1_dense_block(d0):
                dw = min(512, NPC - d0)
                for m in range(8):
                    pm = pmm.tile([P, 512], F32, name=f"apm_{d0}_{m}", tag="pmm", space="PSUM")
                    for k in range(3):
                        kk = (128, 128, 44)[k]
                        nc.tensor.matmul(
                            pm[:, :dw],
                            w1[k][0][:kk, m * P : (m + 1) * P],
                            sc(k, d0, d0 + dw, kk),
                            start=(k == 0), stop=(k == 2),
                        )
                    ev = sb.tile([P, 512], DT, name=f"aev_{d0}_{m}", tag="ev", bufs=6)
                    nc.scalar.activation(
                        ev[:, :dw], pm[:, :dw], LRELU,
                        bias=b1c[:, m : m + 1], alpha=NEG_SLOPE,
                    )
                    if d0 + dw <= HPC:
                        nc.scalar.dma_start(h1T_a[:, m * HPC + d0 : m * HPC + d0 + dw], ev[:, :dw])
                    elif d0 >= HPC:
                        nc.scalar.dma_start(h1T_b[:, m * HPC + d0 - HPC : m * HPC + d0 - HPC + dw], ev[:, :dw])
                    else:
                        cut = HPC - d0
                        nc.scalar.dma_start(h1T_a[:, m * HPC + d0 : m * HPC + HPC], ev[:, :cut])
                        nc.scalar.dma_start(h1T_b[:, m * HPC : m * HPC + dw - cut], ev[:, cut:dw])

            # L1: both halves available immediately -> accumulate lo+hi chunks
            # in one PSUM group per (tile, fc); copy to scratch; dense chases.
            l1_state = {"done": 0}
            for t in range(N_TILES):
                tw = TILE_WIDTHS[t]
                ts = TILE_STARTS[t]
                nch_l = int(sched_lo[t]) // P
                nch_h = int(sched_hi[t]) // P
                xgl = sb.tile([128, nch_l, 300], DT, name=f"xgl_{t}", tag="xg", bufs=3)
                nc.sync.dma_start(xgl[:], xg_lo_d[:, int(coff["lo"][t]) : int(coff["lo"][t]) + nch_l, :])
                xgh = sb.tile([128, nch_h, 300], DT, name=f"xgh_{t}", tag="xg", bufs=3)
                nc.sync.dma_start(xgh[:], xg_hi_d[:, int(coff["hi"][t]) : int(coff["hi"][t]) + nch_h, :])
                for fc in range(3):
                    fw = min(P, 300 - fc * P)
                    pt = pagg.tile([P, P], F32, name=f"pt1_{t}_{fc}",
                                   tag=f"pagg{fc}", space="PSUM", bufs=1)
                    for ci in range(nch_l):
                        nc.tensor.matmul(
                            pt[:fw, :],
                            xgl[:, ci, fc * P : fc * P + fw],
                            s_sb["lo"][:, int(coff["lo"][t]) + ci, :],
                            start=(ci == 0), stop=False,
                        )
                    for ci in range(nch_h):
                        nc.tensor.matmul(
                            pt[:fw, :],
                            xgh[:, ci, fc * P : fc * P + fw],
                            s_sb["hi"][:, int(coff["hi"][t]) + ci, :],
                            start=False, stop=(ci == nch_h - 1),
                        )
                    nc.vector.tensor_copy(sc(fc, ts, ts + tw, fw), pt[:fw, :tw])
                covered = ts + tw
                nblk = covered // 512 if covered < NPC else _ceil_div(NPC, 512)
                while l1_state["done"] < nblk:
                    l1_dense_block(l1_state["done"] * 512)
                    l1_state["done"] += 1

            # ================= dense W2: h1T -> g2 (+AG2) =================
            for bi in range(_ceil_div(NPC, 512)):
                d0 = bi * 512
                dw = min(512, NPC - d0)
                hsb = sb.tile([128, 8, 512], DT, name=f"hsb_{d0}", tag="hg", bufs=3)
                for k in range(8):
                    if d0 + dw <= HPC:
                        nc.sync.dma_start(hsb[:, k, :dw], h1T_a[:, k * HPC + d0 : k * HPC + d0 + dw])
                    elif d0 >= HPC:
                        nc.sync.dma_start(hsb[:, k, :dw], h1T_b[:, k * HPC + d0 - HPC : k * HPC + d0 - HPC + dw])
                    else:
                        cut = HPC - d0
                        nc.sync.dma_start(hsb[:, k, :cut], h1T_a[:, k * HPC + d0 : k * HPC + HPC])
                        nc.sync.dma_start(hsb[:, k, cut:dw], h1T_b[:, k * HPC : k * HPC + dw - cut])
                for m4 in range(_ceil_div(dw, P)):
                    mw = min(P, dw - m4 * P)
                    r0 = d0 + m4 * P
                    pm = pmm.tile([P, 512], F32, name=f"pm2_{d0}_{m4}", tag="pmm", space="PSUM")
                    for k in range(8):
                        nc.tensor.matmul(
                            pm[:mw, :],
                            hsb[:, k, m4 * P : m4 * P + mw],
                            w2[k][0][:, :],
                            start=(k == 0), stop=(k == 7),
                        )
                    ev = sb.tile([P, 512], DT, name=f"ev2_{d0}_{m4}", tag="ev", bufs=6)
                    nc.vector.tensor_copy(ev[:mw, :], pm[:mw, :])
                    if r0 + mw <= HPC:
                        nc.scalar.dma_start(g2a[r0 : r0 + mw, :], ev[:mw, :])
                    elif r0 >= HPC:
                        nc.scalar.dma_start(g2b[r0 - HPC : r0 - HPC + mw, :], ev[:mw, :])
                    else:
                        cut = HPC - r0
                        nc.scalar.dma_start(g2a[r0:HPC, :], ev[:cut, :])
                        nc.scalar.dma_start(g2b[0 : r0 + mw - HPC, :], ev[cut:mw, :])
                    if r0 < HPC <= r0 + mw:
                        ag(g2a, T2a)
            ag(g2b, T2b)

            # ================= L2: agg(g2) +b2+lrelu; W3 -> g3 =================
            d3cb = dense_from_scratch(3, 512, 256, w3, g3a, g3b)
            l2_state = {"on_a": lambda: ag(g3a, T3a), "acted": 0}

            def l2_group_cb(t1):
                covered = TILE_STARTS[t1] + TILE_WIDTHS[t1]
                r0 = l2_state["acted"]
                for fc in range(4):
                    blk = sc(fc, r0, covered)
                    nc.scalar.activation(blk, blk, LRELU,
                                         bias=b2c[:, fc : fc + 1], alpha=NEG_SLOPE)
                l2_state["acted"] = covered
                l2_state["on_a"] = d3cb(
                    covered, l2_state["on_a"],
                    (lambda: ag(g3b, T3b)) if covered >= NPC else None)

            sweep(1, "lo", T2a[:, :], 512, "copy")
            sweep(1, "hi", T2b[:, :], 512, "add", group_cb=l2_group_cb)

            # ================= L3: agg(g3) +b3+lrelu; W4 -> g4 =================
            d4cb = dense_from_scratch(4, 256, 128, w4, g4a, g4b)
            l3_state = {"on_a": lambda: ag(g4a, T4a), "acted": 0}

            def l3_group_cb(t1):
                covered = TILE_STARTS[t1] + TILE_WIDTHS[t1]
                r0 = l3_state["acted"]
                for fc in range(2):
                    blk = sc(fc, r0, covered)
                    nc.scalar.activation(blk, blk, LRELU,
                                         bias=b3c[:, fc : fc + 1], alpha=NEG_SLOPE)
                l3_state["acted"] = covered
                l3_state["on_a"] = d4cb(
                    covered, l3_state["on_a"],
                    (lambda: ag(g4b, T4b)) if covered >= NPC else None)

            sweep(2, "lo", T3a[:, :], 256, "copy")
            sweep(2, "hi", T3b[:, :], 256, "add", group_cb=l3_group_cb)

            # ================= L4: agg(g4) +b4+lrelu (node-major) -> h4 =================
            l4_state = {"evicted": 0}

            def l4_group_cb(t1):
                for t in range(l4_state["evicted"], t1 + 1):
                    tw = TILE_WIDTHS[t]
                    blk = scratch[:tw, H4OFF + t * P : H4OFF + t * P + 128]
                    nc.scalar.activation(blk, blk, LRELU, alpha=NEG_SLOPE)
                    ts = TILE_STARTS[t]
                    if t < TPH:
                        nc.scalar.dma_start(h4a[ts : ts + tw, :], blk)
                    else:
                        nc.scalar.dma_start(h4b[ts - HPC : ts - HPC + tw, :], blk)
                    if t == TPH - 1:
                        ag(h4a, T5a)
                    elif t == N_TILES - 1:
                        ag(h4b, T5b)
                l4_state["evicted"] = t1 + 1

            sweep(3, "lo", T4a[:, :], 128, "copy")
            sweep(3, "hi", T4b[:, :], 128, "add", group_cb=l4_group_cb)

            # ================= L5: agg(h4) -> scratch[0]; W5 (+b5) -> out =================
            l5_state = {"done": 0}

            def l5_group_cb(t1):
                covered = TILE_STARTS[t1] + TILE_WIDTHS[t1]
                nblk = covered // P if covered < NPC else _ceil_div(NPC, P)
                while l5_state["done"] < nblk:
                    d = l5_state["done"]
                    r0 = d * P
                    rw = min(P, NPC - r0)
                    ev = sb.tile([P, 2048], DT, name=f"oev_{d}", tag="oev", bufs=3)
                    for n in range(4):
                        pm = pmm.tile([P, 512], F32, name=f"pm5_{d}_{n}", tag="pmm", space="PSUM")
                        nc.tensor.matmul(
                            pm[:rw, :], sc(0, r0, r0 + rw),
                            w5[0][0][:, n * 512 : (n + 1) * 512],
                            start=True, stop=True,
                        )
                        nc.vector.tensor_tensor(
                            out=ev[:rw, n * 512 : (n + 1) * 512], in0=pm[:rw, :],
                            in1=b5r_sb[:rw, n * 512 : (n + 1) * 512],
                            op=mybir.AluOpType.add,
                        )
                    nc.scalar.dma_start(out_d[r0 : r0 + rw, :], ev[:rw, :])
                    l5_state["done"] += 1

            sweep(4, "lo", T5a[:, :], 128, "copy")
            sweep(4, "hi", T5b[:, :], 128, "add", group_cb=l5_group_cb)

    nc.compile()
    return nc


# ----------------------------------------------------------------------------
# Entry point
# ----------------------------------------------------------------------------

_CACHE = {}


def _run(inputs, trace=False):
    x = np.asarray(inputs["x"], dtype=np.float32)
    edge_index = np.asarray(inputs["edge_index"])
    sched_lo, sched_hi, per_core = _prep(edge_index, x)

    key = (tuple(sched_lo.tolist()), tuple(sched_hi.tolist()))
    if key not in _CACHE:
        _CACHE[key] = _build(sched_lo, sched_hi)
    nc = _CACHE[key]

    common = {}
    for i in range(5):
        common[f"W{i+1}"] = np.ascontiguousarray(
            np.asarray(inputs[f"W{i+1}"], dtype=np.float32).astype(NPDT))
        common[f"b{i+1}"] = np.ascontiguousarray(
            np.asarray(inputs[f"b{i+1}"], dtype=np.float32).reshape(-1, 1))
    common["b4r"] = np.ascontiguousarray(common["b4"].reshape(1, 128).astype(NPDT))
    common["b5r"] = np.ascontiguousarray(
        np.broadcast_to(
            np.asarray(inputs["b5"], dtype=np.float32).reshape(1, 2048), (128, 2048)
        ).astype(NPDT))

    in_maps = [{**common, **per_core[c]} for c in range(NC)]
    res = run_bass_kernel_spmd(nc, in_maps, core_ids=list(range(NC)), trace=trace)
    out = np.empty((N_NODES, 2048), dtype=np.float32)
    for c in range(NC):
        oc = np.asarray(res.results[c]["out"], dtype=np.float32)
        out[c * HPC : (c + 1) * HPC] = oc[:HPC]
        out[HALF + c * HPC : HALF + (c + 1) * HPC] = oc[HPC:]
    return out, res


def kernel(**inputs):
    out, _ = _run(inputs, trace=False)
    return out


# revision 18
# speedup vs baseline: 1.0160x; 1.0160x over previous
"""5-layer DGL-style GraphConv (AwA2Conv) on 8 Trainium2 NeuronCores — v3.

Math per layer (norm='both'):
    out = D_in^{-1/2} A D_out^{-1/2} (h) @ W + b     (+ leaky_relu except last)

Per-edge weight w_e = dinv_out[src]*dinv_in[dst] folded into block-sparse
"S" chunks (128-slot x 128-dst); aggregation = PE matmuls over gathered
edge rows, at width min(Fin, Fout) per layer. Aggregated activations live
feature-major in a resident flat SBUF scratch so the following dense
matmul reads them directly.

Perf structure (bottlenecks, in order): GpSimd dma_gather descriptor
generation is ~6.7ns/index and strictly serial — it bounds each
aggregation layer; the 2 AllGathers per exchange serialize on the
collective engine; HBM traffic is the third constraint.
- gathers merged to <=1024 indices per call (>=1040 wedges the device)
- S matrices (identical across layers) loaded ONCE into resident SBUF
- xg (layer-1 pregathered rows) shipped at true width 300, partition-major
- dense outputs split per node-half so AG_a fires early; lo-sweep
  aggregation stages into SBUF so all table-A work overlaps AG_b
- dense layers chase aggregation tiles; L1 aggregates straight into PSUM
  (both halves available immediately — no staging)
"""

import os

import numpy as np
import ml_dtypes

# recover cleanly if a previous run left the NeuronCores wedged
os.environ.setdefault("NEURON_RT_RESET_CORES", "1")

import concourse.bass as bass
import concourse.bacc as bacc
import concourse.mybir as mybir
import concourse.tile as tile
from concourse.bass_utils import run_bass_kernel_spmd

N_NODES = 50000
N_EDGES = 250000
NC = 8
NPC = N_NODES // NC      # 6250 nodes per core
HALF = 25000             # lo/hi source split (table A/B)
HPC = HALF // NC         # 3125 nodes per core per half
P = 128
TPH = 25                 # dst tiles per half (24x128 + 1x53)
N_TILES = 2 * TPH        # 50 dst tiles per core
DIMS = [300, 1024, 512, 256, 128, 2048]
NEG_SLOPE = 0.01
GRAN = 128               # slot granularity (chunk-aligned segments)
GCAP = 1024              # dma_gather hard cap on num_idxs

F32 = mybir.dt.float32
BF16 = mybir.dt.bfloat16
DT = BF16
NPDT = ml_dtypes.bfloat16
I16 = mybir.dt.int16
LRELU = mybir.ActivationFunctionType.Lrelu

H4OFF = 2 * NPC          # h4 node-major stage overlay offset in scratch


def _ceil_div(a, b):
    return (a + b - 1) // b


def _tile_start(t):
    return (t // TPH) * HPC + (t % TPH) * P


def _tile_width(t):
    return HPC - (TPH - 1) * P if (t % TPH) == TPH - 1 else P


TILE_STARTS = [_tile_start(t) for t in range(N_TILES)]
TILE_WIDTHS = [_tile_width(t) for t in range(N_TILES)]


def _make_groups(sched):
    """Pack consecutive tiles into gather groups of <= GCAP slots."""
    groups = []
    cur, cur_slots, s0 = [], 0, 0
    off = 0
    for t in range(N_TILES):
        s = int(sched[t])
        if cur and cur_slots + s > GCAP:
            groups.append((cur, s0, cur_slots))
            cur, cur_slots, s0 = [], 0, off
        cur.append(t)
        cur_slots += s
        off += s
    if cur:
        groups.append((cur, s0, cur_slots))
    assert all(s <= GCAP for _, _, s in groups)
    return groups


# ----------------------------------------------------------------------------
# Host-side graph preprocessing
# ----------------------------------------------------------------------------

def _prep(edge_index, x):
    src = np.asarray(edge_index[0], dtype=np.int64)
    dst = np.asarray(edge_index[1], dtype=np.int64)
    out_deg = np.bincount(src, minlength=N_NODES).astype(np.float32)
    in_deg = np.bincount(dst, minlength=N_NODES).astype(np.float32)
    dinv_out = 1.0 / np.sqrt(np.maximum(out_deg, 1.0))
    dinv_in = 1.0 / np.sqrt(np.maximum(in_deg, 1.0))
    w = (dinv_out[src] * dinv_in[dst]).astype(np.float32)
    xb = np.asarray(x, dtype=np.float32)

    d_half = dst // HALF
    d_rem = dst % HALF
    d_core = d_rem // HPC
    d_with = d_rem % HPC
    d_pos = d_with + d_half * HPC
    d_tile = d_half * TPH + np.minimum(d_with // P, TPH - 1)
    hi = (src >= HALF).astype(np.int64)

    key = (d_core * N_TILES + d_tile) * 2 + hi
    order = np.lexsort((src, key))
    src_s, w_s, pos_s, key_s = src[order], w[order], d_pos[order], key[order]
    bounds = np.searchsorted(key_s, np.arange(NC * N_TILES * 2 + 1))

    n_lo = np.zeros((NC, N_TILES), dtype=np.int64)
    n_hi = np.zeros((NC, N_TILES), dtype=np.int64)
    for c in range(NC):
        for t in range(N_TILES):
            k = (c * N_TILES + t) * 2
            n_lo[c, t] = bounds[k + 1] - bounds[k]
            n_hi[c, t] = bounds[k + 2] - bounds[k + 1]

    sched_lo = np.maximum(
        np.ceil(n_lo.max(axis=0) / GRAN).astype(np.int64), 1) * GRAN
    sched_hi = np.maximum(
        np.ceil(n_hi.max(axis=0) / GRAN).astype(np.int64), 1) * GRAN

    soff = {"lo": np.concatenate([[0], np.cumsum(sched_lo)]).astype(int),
            "hi": np.concatenate([[0], np.cumsum(sched_hi)]).astype(int)}
    scheds = {"lo": sched_lo, "hi": sched_hi}

    per_core = []
    for c in range(NC):
        pc = {}
        for nm, hbit in (("lo", 0), ("hi", 1)):
            slots = int(soff[nm][-1])
            nch = slots // P
            idx_flat = np.zeros(slots, dtype=np.int16)
            s_flat = np.zeros((slots, P), dtype=np.float32)
            xg_flat = np.zeros((slots, 300), dtype=NPDT)
            for t in range(N_TILES):
                k = (c * N_TILES + t) * 2 + hbit
                a, b_ = int(bounds[k]), int(bounds[k + 1])
                ne = b_ - a
                j0 = int(soff[nm][t])
                assert ne <= int(scheds[nm][t])
                if ne == 0:
                    continue
                srcs = src_s[a:b_] - hbit * HALF
                idx_flat[j0 : j0 + ne] = srcs.astype(np.int16)
                cols = (pos_s[a:b_] - TILE_STARTS[t]).astype(np.int64)
                s_flat[j0 + np.arange(ne), cols] = w_s[a:b_]
                xg_flat[j0 : j0 + ne, :] = xb[src_s[a:b_]].astype(NPDT)
            pc[f"idx_{nm}"] = np.ascontiguousarray(
                np.tile(idx_flat.reshape(-1, 16).T, (8, 1)))
            pc[f"s_{nm}"] = np.ascontiguousarray(
                s_flat.reshape(nch, P, P).transpose(1, 0, 2).astype(NPDT))
            pc[f"xg_{nm}"] = np.ascontiguousarray(
                xg_flat.reshape(nch, P, 300).transpose(1, 0, 2))
        per_core.append(pc)
    return sched_lo, sched_hi, per_core


# ----------------------------------------------------------------------------
# Bass program builder (depends only on the schedules)
# ----------------------------------------------------------------------------

def _build(sched_lo, sched_hi):
    nc = bacc.Bacc("TRN2")
    scheds = {"lo": sched_lo, "hi": sched_hi}
    coff = {nm: np.concatenate([[0], np.cumsum(scheds[nm] // P)]).astype(int)
            for nm in ("lo", "hi")}
    ioff = {nm: np.concatenate([[0], np.cumsum(scheds[nm] // 16)]).astype(int)
            for nm in ("lo", "hi")}
    groups = {nm: _make_groups(scheds[nm]) for nm in ("lo", "hi")}
    totc = {nm: int(coff[nm][-1]) for nm in ("lo", "hi")}
    toti = {nm: int(ioff[nm][-1]) for nm in ("lo", "hi")}

    xg_lo_d = nc.declare_dram_parameter("xg_lo", [128, totc["lo"], 300], DT, isOutput=False)
    xg_hi_d = nc.declare_dram_parameter("xg_hi", [128, totc["hi"], 300], DT, isOutput=False)
    s_lo_d = nc.declare_dram_parameter("s_lo", [128, totc["lo"], P], DT, isOutput=False)
    s_hi_d = nc.declare_dram_parameter("s_hi", [128, totc["hi"], P], DT, isOutput=False)
    idx_lo_d = nc.declare_dram_parameter("idx_lo", [128, toti["lo"]], I16, isOutput=False)
    idx_hi_d = nc.declare_dram_parameter("idx_hi", [128, toti["hi"]], I16, isOutput=False)
    Ws, bs = [], []
    for i in range(5):
        fi, fo = DIMS[i], DIMS[i + 1]
        Ws.append(nc.declare_dram_parameter(f"W{i+1}", [fi, fo], DT, isOutput=False))
        bs.append(nc.declare_dram_parameter(f"b{i+1}", [fo, 1], F32, isOutput=False))
    b4r_d = nc.declare_dram_parameter("b4r", [1, 128], DT, isOutput=False)
    b5r_d = nc.declare_dram_parameter("b5r", [128, 2048], DT, isOutput=False)
    out_d = nc.declare_dram_parameter("out", [NPC, 2048], DT, isOutput=True)

    with tile.TileContext(nc) as tc:
        with (
            tc.tile_pool(name="dram", bufs=1, space="DRAM") as dram,
            tc.tile_pool(name="cpool", bufs=1) as cpool,
            tc.tile_pool(name="sb", bufs=2) as sb,
            tc.tile_pool(name="pagg", bufs=1, space="PSUM") as pagg,
            tc.tile_pool(name="pmm", bufs=4, space="PSUM") as pmm,
        ):
            # ---- internal DRAM ----
            h1T_a = dram.tile([128, 8 * HPC], DT)
            h1T_b = dram.tile([128, 8 * HPC], DT)
            g2a = dram.tile([HPC, 512], DT)
            g2b = dram.tile([HPC, 512], DT)
            g3a = dram.tile([HPC, 256], DT)
            g3b = dram.tile([HPC, 256], DT)
            g4a = dram.tile([HPC, 128], DT)
            g4b = dram.tile([HPC, 128], DT)
            h4a = dram.tile([HPC, 128], DT)
            h4b = dram.tile([HPC, 128], DT)
            T2a = dram.tile([HALF, 512], DT, addr_space="Shared")
            T2b = dram.tile([HALF, 512], DT, addr_space="Shared")
            T3a = dram.tile([HALF, 256], DT, addr_space="Shared")
            T3b = dram.tile([HALF, 256], DT, addr_space="Shared")
            T4a = dram.tile([HALF, 128], DT, addr_space="Shared")
            T4b = dram.tile([HALF, 128], DT, addr_space="Shared")
            T5a = dram.tile([HALF, 128], DT, addr_space="Shared")
            T5b = dram.tile([HALF, 128], DT, addr_space="Shared")

            # ---- resident SBUF ----
            # flat scratch: chunks k at cols [k*NPC,(k+1)*NPC); h4 node-major
            # stage overlays cols [H4OFF, H4OFF + 50*128) (chunks 2-3 region,
            # free during L4 which only reads chunks 0-1)
            scratch = cpool.tile([P, 4 * NPC], DT, name="scratch")
            s_sb = {}
            s_sb["lo"] = cpool.tile([128, totc["lo"], P], DT, name="slores")
            nc.sync.dma_start(s_sb["lo"][:], s_lo_d[:])
            s_sb["hi"] = cpool.tile([128, totc["hi"], P], DT, name="shires")
            nc.sync.dma_start(s_sb["hi"][:], s_hi_d[:])
            ones_sb = cpool.tile([1, 128], DT, name="ones")
            nc.any.memset(ones_sb[:], 1.0)
            b4r_sb = cpool.tile([1, 128], DT, name="b4rsb")
            nc.sync.dma_start(b4r_sb[:], b4r_d[:])
            b5r_sb = cpool.tile([128, 2048], DT, name="b5rsb")
            nc.sync.dma_start(b5r_sb[:], b5r_d[:])
            idx_sb = {}
            idx_sb["lo"] = cpool.tile([128, toti["lo"]], I16, name="idxlo")
            nc.sync.dma_start(idx_sb["lo"][:], idx_lo_d[:])
            idx_sb["hi"] = cpool.tile([128, toti["hi"]], I16, name="idxhi")
            nc.sync.dma_start(idx_sb["hi"][:], idx_hi_d[:])

            rg = [list(range(NC))]

            def load_w(i):
                fi, fo = DIMS[i], DIMS[i + 1]
                ks = []
                for k in range(_ceil_div(fi, P)):
                    kk = min(P, fi - k * P)
                    t_ = cpool.tile([P, fo], DT, name=f"w{i}_{k}")
                    nc.sync.dma_start(t_[:kk, :], Ws[i][k * P : k * P + kk, :])
                    ks.append((t_, kk))
                return ks

            def load_bcol(i):
                fo = DIMS[i + 1]
                t_ = cpool.tile([P, 16], F32, name=f"bc{i}")
                for m in range(_ceil_div(fo, P)):
                    mm = min(P, fo - m * P)
                    nc.sync.dma_start(t_[:mm, m : m + 1], bs[i][m * P : m * P + mm, :])
                return t_

            w1 = load_w(0)
            w2 = load_w(1)
            w3 = load_w(2)
            w4 = load_w(3)
            w5 = load_w(4)
            b1c = load_bcol(0)
            b2c = load_bcol(1)
            b3c = load_bcol(2)

            def ag(src_d, dst_d):
                nc.gpsimd.collective_compute(
                    "AllGather", mybir.AluOpType.bypass, replica_groups=rg,
                    ins=[src_d[:].opt()], outs=[dst_d[:].opt()],
                )

            def sc(k, c0, c1, pw=P):
                return scratch[:pw, k * NPC + c0 : k * NPC + c1]

            # ---------------- aggregation sweeps (layers 2-5) ----------------
            def sweep(layer, nm, tab, fa, mode, group_cb=None):
                """One half-sweep of a layer's aggregation via dma_gather.

                mode 'copy': psum -> stage; 'add': stage += psum.
                Stage is scratch chunks (feature-major) or the h4 overlay
                (layer 3, node-major). group_cb(last_tile) after each group.
                """
                nfc = _ceil_div(fa, P)
                sched = scheds[nm]
                ssb = s_sb[nm]
                for tiles, s0, slots in groups[nm]:
                    t0, t1 = tiles[0], tiles[-1]
                    c0 = int(coff[nm][t0])
                    gc = int(coff[nm][t1 + 1]) - c0
                    hg = sb.tile([128, gc, fa], DT, name=f"hg_{layer}_{nm}_{t0}", tag="hg", bufs=3)
                    nc.gpsimd.dma_gather(
                        hg[:], tab,
                        idx_sb[nm][:, int(ioff[nm][t0]) : int(ioff[nm][t1 + 1])],
                        slots, slots, fa,
                    )
                    for t in tiles:
                        tw = TILE_WIDTHS[t]
                        ts = TILE_STARTS[t]
                        ca = int(coff[nm][t]) - c0
                        cg = int(coff[nm][t])
                        nch = int(sched[t]) // P
                        if layer == 3:  # node-major
                            pt = pagg.tile([P, P], F32, name=f"pt_{layer}_{nm}_{t}",
                                           tag="pagg0", space="PSUM", bufs=1)
                            last = nch - 1 if mode == "copy" else nch
                            for ci in range(nch):
                                nc.tensor.matmul(
                                    pt[:, :fa], ssb[:, cg + ci, :], hg[:, ca + ci, :fa],
                                    start=(ci == 0), stop=(ci == last),
                                )
                            if mode == "add":  # bias row closes the group
                                nc.tensor.matmul(
                                    pt[:, :fa], ones_sb[:1, :], b4r_sb[:1, :fa],
                                    start=False, stop=True,
                                )
                            dst = scratch[:tw, H4OFF + t * P : H4OFF + t * P + fa]
                            if mode == "copy":
                                nc.vector.tensor_copy(dst, pt[:tw, :fa])
                            else:
                                nc.vector.tensor_tensor(
                                    out=dst, in0=pt[:tw, :fa], in1=dst,
                                    op=mybir.AluOpType.add,
                                )
                        else:
                            for fc in range(nfc):
                                fw = min(P, fa - fc * P)
                                pt = pagg.tile([P, P], F32, name=f"pt_{layer}_{nm}_{t}_{fc}",
                                               tag=f"pagg{fc}", space="PSUM", bufs=1)
                                for ci in range(nch):
                                    nc.tensor.matmul(
                                        pt[:fw, :],
                                        hg[:, ca + ci, fc * P : fc * P + fw],
                                        ssb[:, cg + ci, :],
                                        start=(ci == 0), stop=(ci == nch - 1),
                                    )
                                dst = sc(fc, ts, ts + tw, fw)
                                if mode == "copy":
                                    nc.vector.tensor_copy(dst, pt[:fw, :tw])
                                else:
                                    nc.vector.tensor_tensor(
                                        out=dst, in0=pt[:fw, :tw], in1=dst,
                                        op=mybir.AluOpType.add,
                                    )
                    if group_cb is not None:
                        group_cb(t1)

            # ---------------- dense helpers ----------------
            def dense_from_scratch(li, fi, fo, wt, ga, gb):
                """g[n,:] = scratch[:, :, n]^T @ W, evicted to ga/gb by half."""
                nk = _ceil_div(fi, P)
                state = {"done": 0}

                def emit_block(d0):
                    dw = min(512, NPC - d0)
                    for m4 in range(_ceil_div(dw, P)):
                        mw = min(P, dw - m4 * P)
                        r0 = d0 + m4 * P
                        pm = pmm.tile([P, fo], F32, name=f"pm_{li}_{d0}_{m4}",
                                      tag="pmm", space="PSUM")
                        for k in range(nk):
                            kk = min(P, fi - k * P)
                            nc.tensor.matmul(
                                pm[:mw, :fo],
                                sc(k, r0, r0 + mw, kk),
                                wt[k][0][:kk, :fo],
                                start=(k == 0), stop=(k == nk - 1),
                            )
                        ev = sb.tile([P, fo], DT, name=f"ev_{li}_{d0}_{m4}", tag="ev", bufs=4)
                        nc.vector.tensor_copy(ev[:mw, :fo], pm[:mw, :fo])
                        if r0 + mw <= HPC:
                            nc.scalar.dma_start(ga[r0 : r0 + mw, :fo], ev[:mw, :fo])
                        elif r0 >= HPC:
                            nc.scalar.dma_start(gb[r0 - HPC : r0 - HPC + mw, :fo], ev[:mw, :fo])
                        else:
                            cut = HPC - r0
                            nc.scalar.dma_start(ga[r0 : HPC, :fo], ev[:cut, :fo])
                            nc.scalar.dma_start(gb[0 : r0 + mw - HPC, :fo], ev[cut:mw, :fo])

                def cb(covered, on_a=None, on_b=None):
                    nblk = covered // 512 if covered < NPC else _ceil_div(NPC, 512)
                    while state["done"] < nblk:
                        emit_block(state["done"] * 512)
                        state["done"] += 1
                        if state["done"] * 512 >= HPC and on_a is not None:
                            on_a()
                            on_a = None
                    if covered >= NPC and on_b is not None:
                        on_b()
                    return on_a

                return cb

            # ================= L1: agg(x) -> scratch; dense W1 -> h1T =================
            def l1_dense_block(d0):
                dw = min(512, NPC - d0)
                for m in range(8):
                    pm = pmm.tile([P, 512], F32, name=f"apm_{d0}_{m}", tag="pmm", space="PSUM")
                    for k in range(3):
                        kk = (128, 128, 44)[k]
                        nc.tensor.matmul(
                            pm[:, :dw],
                            w1[k][0][:kk, m * P : (m + 1) * P],
                            sc(k, d0, d0 + dw, kk),
                            start=(k == 0), stop=(k == 2),
                        )
                    ev = sb.tile([P, 512], DT, name=f"aev_{d0}_{m}", tag="ev", bufs=4)
                    nc.scalar.activation(
                        ev[:, :dw], pm[:, :dw], LRELU,
                        bias=b1c[:, m : m + 1], alpha=NEG_SLOPE,
                    )
                    if d0 + dw <= HPC:
                        nc.scalar.dma_start(h1T_a[:, m * HPC + d0 : m * HPC + d0 + dw], ev[:, :dw])
                    elif d0 >= HPC:
                        nc.scalar.dma_start(h1T_b[:, m * HPC + d0 - HPC : m * HPC + d0 - HPC + dw], ev[:, :dw])
                    else:
                        cut = HPC - d0
                        nc.scalar.dma_start(h1T_a[:, m * HPC + d0 : m * HPC + HPC], ev[:, :cut])
                        nc.scalar.dma_start(h1T_b[:, m * HPC : m * HPC + dw - cut], ev[:, cut:dw])

            # L1: both halves available immediately -> accumulate lo+hi chunks
            # in one PSUM group per (tile, fc); copy to scratch; dense chases.
            l1_state = {"done": 0}
            for t in range(N_TILES):
                tw = TILE_WIDTHS[t]
                ts = TILE_STARTS[t]
                nch_l = int(sched_lo[t]) // P
                nch_h = int(sched_hi[t]) // P
                xgl = sb.tile([128, nch_l, 300], DT, name=f"xgl_{t}", tag="xg", bufs=3)
                nc.sync.dma_start(xgl[:], xg_lo_d[:, int(coff["lo"][t]) : int(coff["lo"][t]) + nch_l, :])
                xgh = sb.tile([128, nch_h, 300], DT, name=f"xgh_{t}", tag="xg", bufs=3)
                nc.sync.dma_start(xgh[:], xg_hi_d[:, int(coff["hi"][t]) : int(coff["hi"][t]) + nch_h, :])
                for fc in range(3):
                    fw = min(P, 300 - fc * P)
                    pt = pagg.tile([P, P], F32, name=f"pt1_{t}_{fc}",
                                   tag=f"pagg{fc}", space="PSUM", bufs=1)
                    for ci in range(nch_l):
                        nc.tensor.matmul(
                            pt[:fw, :],
                            xgl[:, ci, fc * P : fc * P + fw],
                            s_sb["lo"][:, int(coff["lo"][t]) + ci, :],
                            start=(ci == 0), stop=False,
                        )
                    for ci in range(nch_h):
                        nc.tensor.matmul(
                            pt[:fw, :],
                            xgh[:, ci, fc * P : fc * P + fw],
                            s_sb["hi"][:, int(coff["hi"][t]) + ci, :],
                            start=False, stop=(ci == nch_h - 1),
                        )
                    nc.vector.tensor_copy(sc(fc, ts, ts + tw, fw), pt[:fw, :tw])
                covered = ts + tw
                nblk = covered // 512 if covered < NPC else _ceil_div(NPC, 512)
                while l1_state["done"] < nblk:
                    l1_dense_block(l1_state["done"] * 512)
                    l1_state["done"] += 1

            # ================= dense W2: h1T -> g2 (+AG2) =================
            for bi in range(_ceil_div(NPC, 512)):
                d0 = bi * 512
                dw = min(512, NPC - d0)
                hsb = sb.tile([128, 8, 512], DT, name=f"hsb_{d0}", tag="hg", bufs=3)
                for k in range(8):
                    if d0 + dw <= HPC:
                        nc.sync.dma_start(hsb[:, k, :dw], h1T_a[:, k * HPC + d0 : k * HPC + d0 + dw])
                    elif d0 >= HPC:
                        nc.sync.dma_start(hsb[:, k, :dw], h1T_b[:, k * HPC + d0 - HPC : k * HPC + d0 - HPC + dw])
                    else:
                        cut = HPC - d0
                        nc.sync.dma_start(hsb[:, k, :cut], h1T_a[:, k * HPC + d0 : k * HPC + HPC])
                        nc.sync.dma_start(hsb[:, k, cut:dw], h1T_b[:, k * HPC : k * HPC + dw - cut])
                for m4 in range(_ceil_div(dw, P)):
                    mw = min(P, dw - m4 * P)
                    r0 = d0 + m4 * P
                    pm = pmm.tile([P, 512], F32, name=f"pm2_{d0}_{m4}", tag="pmm", space="PSUM")
                    for k in range(8):
                        nc.tensor.matmul(
                            pm[:mw, :],
                            hsb[:, k, m4 * P : m4 * P + mw],
                            w2[k][0][:, :],
                            start=(k == 0), stop=(k == 7),
                        )
                    ev = sb.tile([P, 512], DT, name=f"ev2_{d0}_{m4}", tag="ev", bufs=4)
                    nc.vector.tensor_copy(ev[:mw, :], pm[:mw, :])
                    if r0 + mw <= HPC:
                        nc.scalar.dma_start(g2a[r0 : r0 + mw, :], ev[:mw, :])
                    elif r0 >= HPC:
                        nc.scalar.dma_start(g2b[r0 - HPC : r0 - HPC + mw, :], ev[:mw, :])
                    else:
                        cut = HPC - r0
                        nc.scalar.dma_start(g2a[r0:HPC, :], ev[:cut, :])
                        nc.scalar.dma_start(g2b[0 : r0 + mw - HPC, :], ev[cut:mw, :])
                    if r0 < HPC <= r0 + mw:
                        ag(g2a, T2a)
            ag(g2b, T2b)

            # ================= L2: agg(g2) +b2+lrelu; W3 -> g3 =================
            d3cb = dense_from_scratch(3, 512, 256, w3, g3a, g3b)
            l2_state = {"on_a": lambda: ag(g3a, T3a), "acted": 0}

            def l2_group_cb(t1):
                covered = TILE_STARTS[t1] + TILE_WIDTHS[t1]
                r0 = l2_state["acted"]
                for fc in range(4):
                    blk = sc(fc, r0, covered)
                    nc.scalar.activation(blk, blk, LRELU,
                                         bias=b2c[:, fc : fc + 1], alpha=NEG_SLOPE)
                l2_state["acted"] = covered
                l2_state["on_a"] = d3cb(
                    covered, l2_state["on_a"],
                    (lambda: ag(g3b, T3b)) if covered >= NPC else None)

            sweep(1, "lo", T2a[:, :], 512, "copy")
            sweep(1, "hi", T2b[:, :], 512, "add", group_cb=l2_group_cb)

            # ================= L3: agg(g3) +b3+lrelu; W4 -> g4 =================
            d4cb = dense_from_scratch(4, 256, 128, w4, g4a, g4b)
            l3_state = {"on_a": lambda: ag(g4a, T4a), "acted": 0}

            def l3_group_cb(t1):
                covered = TILE_STARTS[t1] + TILE_WIDTHS[t1]
                r0 = l3_state["acted"]
                for fc in range(2):
                    blk = sc(fc, r0, covered)
                    nc.scalar.activation(blk, blk, LRELU,
                                         bias=b3c[:, fc : fc + 1], alpha=NEG_SLOPE)
                l3_state["acted"] = covered
                l3_state["on_a"] = d4cb(
                    covered, l3_state["on_a"],
                    (lambda: ag(g4b, T4b)) if covered >= NPC else None)

            sweep(2, "lo", T3a[:, :], 256, "copy")
            sweep(2, "hi", T3b[:, :], 256, "add", group_cb=l3_group_cb)

            # ================= L4: agg(g4) +b4+lrelu (node-major) -> h4 =================
            l4_state = {"evicted": 0}

            def l4_group_cb(t1):
                for t in range(l4_state["evicted"], t1 + 1):
                    tw = TILE_WIDTHS[t]
                    blk = scratch[:tw, H4OFF + t * P : H4OFF + t * P + 128]
                    nc.scalar.activation(blk, blk, LRELU, alpha=NEG_SLOPE)
                    ts = TILE_STARTS[t]
                    if t < TPH:
                        nc.scalar.dma_start(h4a[ts : ts + tw, :], blk)
                    else:
                        nc.scalar.dma_start(h4b[ts - HPC : ts - HPC + tw, :], blk)
                    if t == TPH - 1:
                        ag(h4a, T5a)
                    elif t == N_TILES - 1:
                        ag(h4b, T5b)
                l4_state["evicted"] = t1 + 1

            sweep(3, "lo", T4a[:, :], 128, "copy")
            sweep(3, "hi", T4b[:, :], 128, "add", group_cb=l4_group_cb)

            # ================= L5: agg(h4) -> scratch[0]; W5 (+b5) -> out =================
            l5_state = {"done": 0}

            def l5_group_cb(t1):
                covered = TILE_STARTS[t1] + TILE_WIDTHS[t1]
                nblk = covered // P if covered < NPC else _ceil_div(NPC, P)
                while l5_state["done"] < nblk:
                    d = l5_state["done"]
                    r0 = d * P
                    rw = min(P, NPC - r0)
                    ev = sb.tile([P, 2048], DT, name=f"oev_{d}", tag="oev", bufs=3)
                    for n in range(4):
                        pm = pmm.tile([P, 512], F32, name=f"pm5_{d}_{n}", tag="pmm", space="PSUM")
                        nc.tensor.matmul(
                            pm[:rw, :], sc(0, r0, r0 + rw),
                            w5[0][0][:, n * 512 : (n + 1) * 512],
                            start=True, stop=True,
                        )
                        nc.vector.tensor_tensor(
                            out=ev[:rw, n * 512 : (n + 1) * 512], in0=pm[:rw, :],
                            in1=b5r_sb[:rw, n * 512 : (n + 1) * 512],
                            op=mybir.AluOpType.add,
                        )
                    nc.scalar.dma_start(out_d[r0 : r0 + rw, :], ev[:rw, :])
                    l5_state["done"] += 1

            sweep(4, "lo", T5a[:, :], 128, "copy")
            sweep(4, "hi", T5b[:, :], 128, "add", group_cb=l5_group_cb)

    nc.compile()
    return nc


# ----------------------------------------------------------------------------
# Entry point
# ----------------------------------------------------------------------------

_CACHE = {}


def _run(inputs, trace=False):
    x = np.asarray(inputs["x"], dtype=np.float32)
    edge_index = np.asarray(inputs["edge_index"])
    sched_lo, sched_hi, per_core = _prep(edge_index, x)

    key = (tuple(sched_lo.tolist()), tuple(sched_hi.tolist()))
    if key not in _CACHE:
        _CACHE[key] = _build(sched_lo, sched_hi)
    nc = _CACHE[key]

    common = {}
    for i in range(5):
        common[f"W{i+1}"] = np.ascontiguousarray(
            np.asarray(inputs[f"W{i+1}"], dtype=np.float32).astype(NPDT))
        common[f"b{i+1}"] = np.ascontiguousarray(
            np.asarray(inputs[f"b{i+1}"], dtype=np.float32).reshape(-1, 1))
    common["b4r"] = np.ascontiguousarray(common["b4"].reshape(1, 128).astype(NPDT))
    common["b5r"] = np.ascontiguousarray(
        np.broadcast_to(
            np.asarray(inputs["b5"], dtype=np.float32).reshape(1, 2048), (128, 2048)
        ).astype(NPDT))

    in_maps = [{**common, **per_core[c]} for c in range(NC)]
    res = run_bass_kernel_spmd(nc, in_maps, core_ids=list(range(NC)), trace=trace)
    out = np.empty((N_NODES, 2048), dtype=np.float32)
    for c in range(NC):
        oc = np.asarray(res.results[c]["out"], dtype=np.float32)
        out[c * HPC : (c + 1) * HPC] = oc[:HPC]
        out[HALF + c * HPC : HALF + (c + 1) * HPC] = oc[HPC:]
    return out, res


def kernel(**inputs):
    out, _ = _run(inputs, trace=False)
    return out
